# revision 42
# baseline (speedup 1.0000x reference)
"""Trainium2 Bass kernel for nn_CascadeTransformerMM (4-layer ternary-GLU cascade).

v9.2: feature-major (transposed) dataflow, packed elementwise batches.
  - Host ternarizes the weights exactly (sign(round(w*scale)) in fp32 RNE)
    and ships them as fp8e4 {-1,0,+1}; no on-device ternarization.
  - Activations live as X[d, t] (d on partitions), SBUF-resident across all
    4 layers.  Up-proj makes U,V as [f, t] (weights stationary, q moving);
    [f, t] feeds the down-proj directly: zero activation transposes.
  - Cross-partition stats via gpsimd partition_all_reduce; per-token scales
    are broadcast tiles consumed through stride-0 broadcast APs.
  - Elementwise work is batched into [128, 4, 256] packs (g_b, gq, q,
    drains) to amortize the ~300ns fixed per-instruction engine overhead;
    gq magic-rounding runs as a single fused 2-op tensor_scalar on DVE.
  - PSUM co-tenancy: start=True zeroes a whole 2KB bank, so only the first
    accumulation group per bank sets it (U yes / V no; even dj yes / odd no).

Math per layer:
  h = rms_scale * x * rstd;  s = clip(127/(max|h|+1e-5), 1e-3, 1e3)
  q = round(s*h)  (magic-number round, bf16-exact ints)
  U = q@Wg_t; V = q@Wu_t  (ternary fp8 weights, int-exact fp32 accum)
  g_b = silu(U*rs) * V;  s2 = clip(127/(max|g_b|*rs + 1e-5), ...)
  gq = round(g_b * s2*rs);  x += (gq@Wd_t) * (1/s2)

Distribution: data-parallel over batch (8 batches -> 8 cores).
"""

import os
import sys

for _p in ("/opt/trn_rl_repo", "/root/.axon_site/_ro/trn_rl_repo"):
    if os.path.isdir(_p) and _p not in sys.path:
        sys.path.insert(0, _p)

import numpy as np
import ml_dtypes
from contextlib import ExitStack

import concourse.bass as bass
import concourse.mybir as mybir
import concourse.tile as tile
from concourse.bass_isa import ReduceOp
from concourse import library_config, library_overlay
from concourse.bass_utils import run_bass_kernel_spmd

dt = mybir.dt
AF = mybir.ActivationFunctionType
ALU = mybir.AluOpType

MAGIC = float(1.5 * 2**23)

D = 1024
F = 4096
L = 4
NCORES = 8
TOK = 1024

TOKT = 256            # tokens per sweep
NSW = TOK // TOKT     # sweeps per layer
NDJ = D // 128        # 8 d-tiles
NFC = F // 128        # 32 f-chunks
NPK = NFC // 4        # 8 f-packs of 4 chunks
NOCT = 4              # wd octets per quad phase
FP8 = dt.float8e4


def _split_excess_waits(nc, max_waits: int = 1) -> int:
    """walrus in this container rejects >1 sync-wait per instruction; split
    extras into standalone event-semaphore waits on the same engine."""
    n = 0
    for func in nc.m.functions:
        for block in func.blocks:
            changed = False
            out = []
            for inst in block.instructions:
                si = getattr(inst, "sync_info", None)
                if si is not None and si.on_wait and len(si.on_wait) > max_waits:
                    waits = list(si.on_wait)
                    for j, w in enumerate(waits[max_waits:]):
                        out.append(
                            mybir.InstEventSemaphore(
                                name=f"{inst.name}-xw{j}",
                                engine=inst.engine,
                                ins=[],
                                outs=[],
                                sync_info=mybir.SyncInfo(on_wait=[w], on_update=[]),
                            )
                        )
                        n += 1
                    inst.sync_info = mybir.SyncInfo(
                        on_wait=waits[:max_waits], on_update=list(si.on_update)
                    )
                    changed = True
                out.append(inst)
            if changed:
                block.instructions = out
    return n


def _b3(ap, n=4):
    """Broadcast a [128, T] AP across a middle dim of n (stride 0)."""
    return bass.AP(ap.tensor, ap.offset, [ap.ap[0], [0, n], ap.ap[1]])


DEBUG = False


def build(n_cores: int = NCORES) -> bass.Bass:
    nc = bass.Bass(num_devices=n_cores)

    x_ext = nc.declare_dram_parameter("x", [D, TOK], dt.float32, isOutput=False)
    rsc_ext = nc.declare_dram_parameter("rsc", [128, L * NDJ], dt.float32, isOutput=False)
    # chunk-grouped layouts: 4 f-chunks (wg/wu) or one quad-octet (wd) per
    # DMA, 4KB contiguous per partition
    wg_ext = nc.declare_dram_parameter("wg", [L, NPK, 128, 4, NDJ, 128], FP8, isOutput=False)
    wu_ext = nc.declare_dram_parameter("wu", [L, NPK, 128, 4, NDJ, 128], FP8, isOutput=False)
    wd_ext = nc.declare_dram_parameter("wd", [L, 2, NOCT, 128, 4, 8, 128], FP8, isOutput=False)
    out_ext = nc.declare_dram_parameter("out", [D, TOK], dt.float32, isOutput=True)
    if DEBUG:
        dbg_ext = nc.declare_dram_parameter("dbg", [16, 128, TOKT], dt.float32, isOutput=True)

    with tile.TileContext(nc) as tc, ExitStack() as ctx:
        P = ctx.enter_context
        const = P(tc.tile_pool(name="const", bufs=1))
        xpool = P(tc.tile_pool(name="X", bufs=1))
        qpool = P(tc.tile_pool(name="q", bufs=5))
        gbpool = P(tc.tile_pool(name="gb", bufs=10))
        gqpool = P(tc.tile_pool(name="gq", bufs=9))
        t4pool = P(tc.tile_pool(name="t4", bufs=4))
        t0pool = P(tc.tile_pool(name="t0", bufs=3))
        trpool = P(tc.tile_pool(name="tr", bufs=4))
        stpool = P(tc.tile_pool(name="st", bufs=1))
        bcpool = P(tc.tile_pool(name="bc", bufs=3))
        smpool = P(tc.tile_pool(name="sm", bufs=2))
        wgupool = P(tc.tile_pool(name="wgu", bufs=2))
        wdpool = P(tc.tile_pool(name="wd", bufs=4))
        psUV = P(tc.tile_pool(name="psUV", bufs=3, space="PSUM"))
        psX = P(tc.tile_pool(name="psX", bufs=1, space="PSUM"))

        # ---------- constants ----------
        nc.gpsimd.load_library(library_config.attn)
        mag = const.tile([128, 1], dt.float32, tag="mag")
        nc.gpsimd.memset(mag[:], MAGIC)
        rscol = const.tile([128, L * NDJ], dt.float32, tag="rscol")
        nc.sync.dma_start(rscol[:], rsc_ext[:, :])

        def sbc_ap(l, dj):
            return rscol[:, l * NDJ + dj:l * NDJ + dj + 1]

        def tap(i, ap):
            if DEBUG:
                nc.sync.dma_start(dbg_ext[i], ap)

        # ---------- persistent X tiles (packed per quad) ----------
        X = {}
        for Q in range(2):
            for s in range(NSW):
                X[(Q, s)] = xpool.tile([128, 4, TOKT], dt.float32,
                                       tag=f"x{Q}_{s}", name=f"x{Q}_{s}")

        # down-proj accumulator: [128, 8, TOKT] fp32 = 4 PSUM banks
        xps = psX.tile([128, NDJ, TOKT], dt.float32, tag="xps", name="xps")

        st = {}  # per (l, s) state

        def S(l, s):
            return st.setdefault((l, s), {})

        # ---------- x load ----------
        def emit_xload(s):
            for dj in range(NDJ):
                nc.sync.dma_start(
                    X[(dj // 4, s)][:, dj % 4, :],
                    x_ext[dj * 128:(dj + 1) * 128, s * TOKT:(s + 1) * TOKT])

        # ---------- stats gather (ssq + max|sbc*x|) from X quad ----------
        def emit_stats_quad(l, s, Q):
            """Accumulate S (sum over d of x^2) and M (max over d of |sbc*x|)
            for layer l's phase-1 stats of sweep s; Q=0 then Q=1."""
            d = S(l, s)
            XQ = X[(Q, s)]
            sq4 = t4pool.tile([128, 4, TOKT], dt.float32, tag="t4", name="sq4")
            nc.vector.tensor_tensor(sq4[:], XQ[:], XQ[:], op=ALU.mult)
            if Q == 0:
                d["S"] = smpool.tile([128, TOKT], dt.float32, tag="S", name="Ssum", bufs=4)
                d["M"] = smpool.tile([128, TOKT], dt.float32, tag="M", name="Mmax", bufs=4)
            for dji in range(4):
                if Q == 0 and dji == 0:
                    nc.vector.tensor_scalar(d["S"][:], sq4[:, 0, :], 0.0, None, op0=ALU.add)
                else:
                    nc.vector.tensor_tensor(d["S"][:], d["S"][:], sq4[:, dji, :], op=ALU.add)
            sxs = []
            for dji in range(4):
                dj = Q * 4 + dji
                sx = trpool.tile([128, TOKT], dt.float32, tag="tr", name="sx")
                nc.scalar.activation(sx[:], XQ[:, dji, :], AF.Abs, scale=sbc_ap(l, dj))
                sxs.append((dj, sx))
            for dj, sx in sxs:
                if dj == 0:
                    nc.vector.tensor_scalar(d["M"][:], sx[:], 0.0, None, op0=ALU.max)
                else:
                    nc.vector.tensor_tensor(d["M"][:], d["M"][:], sx[:], op=ALU.max)

        # ---------- phase 1: rms + act-quant scales ----------
        def emit_phase1(l, s):
            d = S(l, s)
            nc.gpsimd.partition_all_reduce(d["S"][:], d["S"][:], 128, ReduceOp.add)
            nc.gpsimd.partition_all_reduce(d["M"][:], d["M"][:], 128, ReduceOp.max)
            ms = stpool.tile([128, TOKT], dt.float32, tag="st1", name="ms")
            rt = stpool.tile([128, TOKT], dt.float32, tag="st2", name="rt")
            rstd = stpool.tile([128, TOKT], dt.float32, tag="st3", name="rstd")
            nwt = stpool.tile([128, TOKT], dt.float32, tag="st4", name="nwt")
            nc.vector.tensor_scalar(ms[:], d["S"][:], 1.0 / D, 1e-6, op0=ALU.mult, op1=ALU.add)
            nc.scalar.activation(rt[:], ms[:], AF.Sqrt)
            nc.vector.reciprocal(rstd[:], rt[:])
            # Newton step refines the Sqrt-LUT rsqrt (v8-proven)
            nc.vector.tensor_tensor(nwt[:], rstd[:], rstd[:], op=ALU.mult)
            nc.vector.tensor_tensor(nwt[:], nwt[:], ms[:], op=ALU.mult)
            nc.vector.tensor_scalar(nwt[:], nwt[:], -0.5, 1.5, op0=ALU.mult, op1=ALU.add)
            nc.vector.tensor_tensor(rstd[:], rstd[:], nwt[:], op=ALU.mult)
            maxh = stpool.tile([128, TOKT], dt.float32, tag="st5", name="maxh")
            sr = stpool.tile([128, TOKT], dt.float32, tag="st6", name="sr")
            nc.vector.tensor_tensor(maxh[:], d["M"][:], rstd[:], op=ALU.mult)
            nc.vector.tensor_scalar(maxh[:], maxh[:], 1e-5, None, op0=ALU.add)
            nc.vector.reciprocal(sr[:], maxh[:])
            nc.vector.tensor_scalar(sr[:], sr[:], 127.0, 1e3, op0=ALU.mult, op1=ALU.min)
            nc.vector.tensor_scalar(sr[:], sr[:], 1e-3, None, op0=ALU.max)
            c1 = bcpool.tile([128, TOKT], dt.float32, tag="c1", name="c1bc")
            rs = bcpool.tile([128, TOKT], dt.float32, tag="rs", name="rsbc")
            nc.vector.tensor_tensor(c1[:], sr[:], rstd[:], op=ALU.mult)
            # rs = 1/s recip-free: clip(maxh/127, 1e-3, 1e3) (maxh incl +1e-5)
            nc.vector.tensor_scalar(rs[:], maxh[:], 1.0 / 127.0, 1e-3, op0=ALU.mult, op1=ALU.max)
            nc.vector.tensor_scalar(rs[:], rs[:], 1e3, None, op0=ALU.min)
            d["c1"], d["rs"] = c1, rs
            if (l, s) == (0, 0):
                tap(0, d["S"][:]); tap(1, d["M"][:]); tap(2, c1[:]); tap(3, rs[:])

        # ---------- q pass (packed per quad; A = mult+magic1, B = round) ----------
        def emit_qpass_a(l, s):
            d = S(l, s)
            t04s = []
            for Q in range(2):
                t04 = t4pool.tile([128, 4, TOKT], dt.float32, tag="qt4", name="t04", bufs=4)
                nc.vector.tensor_tensor(t04[:], X[(Q, s)][:], _b3(d["c1"][:]), op=ALU.mult)
                for dji in range(4):
                    nc.scalar.activation(t04[:, dji, :], t04[:, dji, :], AF.Identity,
                                         scale=sbc_ap(l, Q * 4 + dji), bias=mag[:])
                t04s.append(t04)
            d["t04"] = t04s

        def emit_qpass_b(l, s):
            d = S(l, s)
            qp = []
            for Q in range(2):
                q4 = qpool.tile([128, 4, TOKT], dt.bfloat16, tag="q", name="q4")
                nc.vector.tensor_scalar(q4[:], d["t04"][Q][:], -MAGIC, None, op0=ALU.add)
                qp.append(q4)
            del d["t04"]
            d["qp"] = qp

        def emit_qpass(l, s):
            emit_qpass_a(l, s)
            emit_qpass_b(l, s)

        def q_ap(d, dk):
            return d["qp"][dk // 4][:, dk % 4, :]

        # ---------- up-proj + GLU, one f-chunk (emission-pipelined) ----------
        def flush_gb(d, l, s, keep=0):
            """Emit deferred V drains (gb = silu * V); lag keeps DVE head from
            waiting on the scalar silu round-trip."""
            pend = d.setdefault("pend", [])
            while len(pend) > keep:
                c, uv, t0 = pend.pop(0)
                cg, ci = divmod(c, 4)
                gbp = d["gbp"][cg]
                nc.vector.tensor_tensor(gbp[:, ci, :], t0[:], uv[:, 1, :], op=ALU.mult)
                if ci == 3:
                    ab4 = t4pool.tile([128, 4, TOKT], dt.float32, tag="t4", name="ab4")
                    nc.scalar.activation(ab4[:], gbp[:], AF.Abs)
                    d.setdefault("pendab", []).append((cg, ab4))

        def flush_mx(d, keep=0):
            pendab = d.setdefault("pendab", [])
            while len(pendab) > keep:
                cg, ab4 = pendab.pop(0)
                if cg == 0:
                    d["mx4"] = smpool.tile([128, 4, TOKT], dt.float32, tag="mx4", name="mx4")
                    nc.vector.tensor_scalar(d["mx4"][:], ab4[:], 0.0, None, op0=ALU.max)
                else:
                    nc.vector.tensor_tensor(d["mx4"][:], d["mx4"][:], ab4[:], op=ALU.max)

        def emit_uv_chunk(l, s, c):
            d = S(l, s)
            cg, ci = divmod(c, 4)
            wgu = d.setdefault("wgu", {})
            if cg not in wgu:
                wgt4 = wgupool.tile([128, 4, NDJ, 128], FP8, tag="wg", name="wgt")
                wut4 = wgupool.tile([128, 4, NDJ, 128], FP8, tag="wu", name="wut")
                nc.sync.dma_start(wgt4[:], wg_ext[l, cg])
                nc.sync.dma_start(wut4[:], wu_ext[l, cg])
                wgu[cg] = (wgt4, wut4)
            wgt4, wut4 = wgu[cg]
            wgt = wgt4[:, ci]
            wut = wut4[:, ci]
            uv = psUV.tile([128, 2, TOKT], dt.float32, tag="uv", name="uvps")
            # U and V share one 2KB PSUM bank; start=True zeroes the WHOLE
            # bank, so only U's first matmul sets it.
            for dk in range(NDJ):
                nc.tensor.matmul(uv[:, 0, :], wgt[:, dk, :], q_ap(d, dk),
                                 start=(dk == 0), stop=(dk == NDJ - 1))
                nc.tensor.matmul(uv[:, 1, :], wut[:, dk, :], q_ap(d, dk),
                                 start=False, stop=(dk == NDJ - 1))
            if c % 4 == 0:
                d.setdefault("gbp", []).append(
                    gbpool.tile([128, 4, TOKT], dt.float32, tag="gb", name="gbp"))
            t0 = t0pool.tile([128, TOKT], dt.float32, tag="t0", name="silut")
            nc.vector.tensor_tensor(t0[:], uv[:, 0, :], d["rs"][:], op=ALU.mult)
            nc.scalar.activation(t0[:], t0[:], AF.Silu)
            d.setdefault("pend", []).append((c, uv, t0))
            flush_gb(d, l, s, keep=2)
            flush_mx(d, keep=1)

        # ---------- phase 2: g quant scales ----------
        def emit_phase2(l, s):
            d = S(l, s)
            flush_gb(d, l, s, keep=0)
            flush_mx(d, keep=0)
            mx = stpool.tile([128, TOKT], dt.float32, tag="st1", name="mxf")
            nc.vector.tensor_tensor(mx[:], d["mx4"][:, 0, :], d["mx4"][:, 1, :], op=ALU.max)
            nc.vector.tensor_tensor(mx[:], mx[:], d["mx4"][:, 2, :], op=ALU.max)
            nc.vector.tensor_tensor(mx[:], mx[:], d["mx4"][:, 3, :], op=ALU.max)
            nc.gpsimd.partition_all_reduce(mx[:], mx[:], 128, ReduceOp.max)
            g1 = stpool.tile([128, TOKT], dt.float32, tag="g1", name="g1")
            g2 = stpool.tile([128, TOKT], dt.float32, tag="g2", name="g2")
            nc.vector.tensor_tensor(g1[:], mx[:], d["rs"][:], op=ALU.mult)
            nc.vector.tensor_scalar(g1[:], g1[:], 1e-5, None, op0=ALU.add)
            # rs2p = 1/s2 computed recip-free: clip((g1+1e-5)/127, 1e-3, 1e3)
            rs2p = bcpool.tile([128, TOKT], dt.float32, tag="rs2", name="rs2p")
            nc.vector.tensor_scalar(rs2p[:], g1[:], 1.0 / 127.0, 1e-3, op0=ALU.mult, op1=ALU.max)
            nc.vector.tensor_scalar(rs2p[:], rs2p[:], 1e3, None, op0=ALU.min)
            nc.vector.reciprocal(g2[:], g1[:])
            nc.vector.tensor_scalar(g2[:], g2[:], 127.0, 1e3, op0=ALU.mult, op1=ALU.min)
            nc.vector.tensor_scalar(g2[:], g2[:], 1e-3, None, op0=ALU.max)  # = s2
            c2p = bcpool.tile([128, TOKT], dt.float32, tag="c2", name="c2p")
            nc.vector.tensor_tensor(c2p[:], g2[:], d["rs"][:], op=ALU.mult)
            d["c2p"], d["rs2p"] = c2p, rs2p
            if (l, s) == (0, 0):
                tap(7, mx[:]); tap(8, c2p[:]); tap(9, rs2p[:])

        # ---------- gq rounding (packed, fused DVE magic) ----------
        def emit_gq(l, s):
            d = S(l, s)
            gqp = []
            for pk in range(NPK):
                gbp = d["gbp"][pk]
                nc.vector.tensor_tensor(gbp[:], gbp[:], _b3(d["c2p"][:]), op=ALU.mult)
                g4 = gqpool.tile([128, 4, TOKT], dt.bfloat16, tag="gq", name="gq4")
                nc.vector.tensor_scalar(g4[:], gbp[:], MAGIC, -MAGIC, op0=ALU.add, op1=ALU.add)
                gqp.append(g4)
            d["gqp"] = gqp

        def gq_ap(d, ft):
            return d["gqp"][ft // 4][:, ft % 4, :]

        # ---------- down-proj + residual + next-layer stats ----------
        def emit_wd_oct(l, Q, oct):
            t = wdpool.tile([128, 4, 8, 128], FP8, tag="wd", name="wdt")
            nc.sync.dma_start(t[:], wd_ext[l, Q, oct])
            return t

        def emit_down(l, s, pre_oct0):
            d = S(l, s)
            for Q in range(2):
                for oct in range(NOCT):
                    wdt = pre_oct0 if (Q == 0 and oct == 0) else emit_wd_oct(l, Q, oct)
                    for k in range(8):
                        ft = oct * 8 + k
                        for dji in range(4):
                            dj = Q * 4 + dji
                            # adjacent dj pairs share a PSUM bank: even dj's
                            # first matmul zeroes it, odd dj rides the zero
                            nc.tensor.matmul(
                                xps[:, dj, :], wdt[:, dji, k, :], gq_ap(d, ft),
                                start=(ft == 0 and dj % 2 == 0), stop=(ft == NFC - 1))
                # drain quad: X += xps * rs2p  (batched, bcast AP)
                upd4 = t4pool.tile([128, 4, TOKT], dt.float32, tag="t4", name="upd4")
                nc.vector.tensor_tensor(upd4[:], xps[:, Q * 4:Q * 4 + 4, :],
                                        _b3(d["rs2p"][:]), op=ALU.mult)
                if (s, Q) == (0, 0) and l == 0:
                    tap(10, upd4[:, 0, :])
                nc.vector.tensor_tensor(X[(Q, s)][:], X[(Q, s)][:], upd4[:], op=ALU.add)
                if (s, Q) == (0, 0) and l < 3:
                    tap(11 + l, X[(0, s)][:, 0, :])
                if l + 1 < L:
                    emit_stats_quad(l + 1, s, Q)
                else:
                    for dji in range(4):
                        dj = Q * 4 + dji
                        nc.sync.dma_start(
                            out_ext[dj * 128:(dj + 1) * 128, s * TOKT:(s + 1) * TOKT],
                            X[(Q, s)][:, dji, :])

        # ---------- main ----------
        for s in range(NSW):
            emit_xload(s)
        emit_stats_quad(0, 0, 0)
        emit_stats_quad(0, 0, 1)
        emit_phase1(0, 0)
        emit_qpass(0, 0)
        for s in range(1, NSW):
            emit_stats_quad(0, s, 0)
            emit_stats_quad(0, s, 1)

        for l in range(L):
            for s in range(NSW):
                first = (l == 0 and s == 0)
                nl, ns = (l, s + 1) if s + 1 < NSW else (l + 1, 0)
                for c in range(0 if first else 8, NFC):
                    emit_uv_chunk(l, s, c)
                    if c == 10 and nl < L:
                        emit_phase1(nl, ns)
                    if c == 16 and nl < L:
                        emit_qpass_a(nl, ns)
                    if c == 24 and nl < L:
                        emit_qpass_b(nl, ns)
                emit_phase2(l, s)
                pre_oct0 = emit_wd_oct(l, 0, 0)
                emit_gq(l, s)
                # hide phase-2 + gq latency under pre-issued chunks of next sweep
                if nl < L:
                    for cpre in range(8):
                        emit_uv_chunk(nl, ns, cpre)
                emit_down(l, s, pre_oct0)
                st.pop((l, s), None)

    library_overlay.lower_extended_insts(nc)
    _split_excess_waits(nc)
    return nc


_nc_cache = {}


def _get_nc(key=(NCORES,)):
    if key not in _nc_cache:
        _nc_cache[key] = build(*key)
    return _nc_cache[key]


def _ternarize(w):
    """Exact host-side ternarize: sign(round(w * 127/(max|w|+1e-5))) in
    fp32 RNE, matching ternarize(weight_quant(w)) in the reference."""
    w = np.ascontiguousarray(w, dtype=np.float32)
    m = np.float32(np.abs(w).max())
    s = np.float32(127.0) / (m + np.float32(1e-5))
    t = np.round(w * s)
    return np.clip(t, np.float32(-1.0), np.float32(1.0))


def _pack_weights(wg, wu, wd):
    f8 = ml_dtypes.float8_e4m3
    wg_p = np.empty((L, NPK, 128, 4, NDJ, 128), dtype=f8)
    wu_p = np.empty((L, NPK, 128, 4, NDJ, 128), dtype=f8)
    wd_p = np.empty((L, 2, NOCT, 128, 4, 8, 128), dtype=f8)
    for l in range(L):
        tg = _ternarize(wg[l])   # [D, F]
        tu = _ternarize(wu[l])
        td = _ternarize(wd[l])   # [F, D]
        # [dk, p, cg, ci, m] -> [cg, p, ci, dk, m]
        wg_p[l] = tg.reshape(NDJ, 128, NPK, 4, 128).transpose(2, 1, 3, 0, 4).astype(f8)
        wu_p[l] = tu.reshape(NDJ, 128, NPK, 4, 128).transpose(2, 1, 3, 0, 4).astype(f8)
        # [oct, k, p, Q, dji, m] -> [Q, oct, p, dji, k, m]
        wd_p[l] = td.reshape(NOCT, 8, 128, 2, 4, 128).transpose(3, 0, 2, 4, 1, 5).astype(f8)
    return np.ascontiguousarray(wg_p), np.ascontiguousarray(wu_p), np.ascontiguousarray(wd_p)


def _make_in_maps(x, rs, wg, wu, wd, n_cores=NCORES):
    wg_p, wu_p, wd_p = _pack_weights(wg, wu, wd)
    # rscol[p, l*8+dk] = rms_scale[l, dk*128+p]
    rsc = np.ascontiguousarray(
        rs.reshape(L, NDJ, 128).transpose(2, 0, 1).reshape(128, L * NDJ),
        dtype=np.float32)
    in_maps = []
    for c in range(n_cores):
        in_maps.append({
            "x": np.ascontiguousarray(x[c].T),   # [D, TOK]
            "rsc": rsc,
            "wg": wg_p,
            "wu": wu_p,
            "wd": wd_p,
        })
    return in_maps


def kernel(x, rms_scale, W_g, W_u, W_d):
    """Full-input entry point: shard over batch, run 8-core SPMD, gather."""
    x = np.ascontiguousarray(np.asarray(x, dtype=np.float32))
    rs = np.ascontiguousarray(np.asarray(rms_scale, dtype=np.float32))
    wg = np.ascontiguousarray(np.asarray(W_g, dtype=np.float32))
    wu = np.ascontiguousarray(np.asarray(W_u, dtype=np.float32))
    wd = np.ascontiguousarray(np.asarray(W_d, dtype=np.float32))
    B, Sx, Dx = x.shape
    assert (B, Sx, Dx) == (NCORES, TOK, D), (B, Sx, Dx)
    nc = _get_nc()
    in_maps = _make_in_maps(x, rs, wg, wu, wd)
    res = run_bass_kernel_spmd(nc, in_maps, list(range(NCORES)))
    return np.stack([np.ascontiguousarray(res.results[c]["out"].T)
                     for c in range(NCORES)], axis=0)


# revision 44
# speedup vs baseline: 1.1911x; 1.1911x over previous
"""Trainium2 Bass kernel for nn_CascadeTransformerMM (4-layer ternary-GLU cascade).

v9.2: feature-major (transposed) dataflow, packed elementwise batches.
  - Host ternarizes the weights exactly (sign(round(w*scale)) in fp32 RNE)
    and ships them as fp8e4 {-1,0,+1}; no on-device ternarization.
  - Activations live as X[d, t] (d on partitions), SBUF-resident across all
    4 layers.  Up-proj makes U,V as [f, t] (weights stationary, q moving);
    [f, t] feeds the down-proj directly: zero activation transposes.
  - Cross-partition stats via gpsimd partition_all_reduce; per-token scales
    are broadcast tiles consumed through stride-0 broadcast APs.
  - Elementwise work is batched into [128, 4, 256] packs (g_b, gq, q,
    drains) to amortize the ~300ns fixed per-instruction engine overhead;
    gq magic-rounding runs as a single fused 2-op tensor_scalar on DVE.
  - PSUM co-tenancy: start=True zeroes a whole 2KB bank, so only the first
    accumulation group per bank sets it (U yes / V no; even dj yes / odd no).

Math per layer:
  h = rms_scale * x * rstd;  s = clip(127/(max|h|+1e-5), 1e-3, 1e3)
  q = round(s*h)  (magic-number round, bf16-exact ints)
  U = q@Wg_t; V = q@Wu_t  (ternary fp8 weights, int-exact fp32 accum)
  g_b = silu(U*rs) * V;  s2 = clip(127/(max|g_b|*rs + 1e-5), ...)
  gq = round(g_b * s2*rs);  x += (gq@Wd_t) * (1/s2)

Distribution: data-parallel over batch (8 batches -> 8 cores).
"""

import os
import sys

for _p in ("/opt/trn_rl_repo", "/root/.axon_site/_ro/trn_rl_repo"):
    if os.path.isdir(_p) and _p not in sys.path:
        sys.path.insert(0, _p)

import numpy as np
import ml_dtypes
from contextlib import ExitStack

import concourse.bass as bass
import concourse.mybir as mybir
import concourse.tile as tile
from concourse.bass_isa import ReduceOp
from concourse import library_config, library_overlay
from concourse.bass_utils import run_bass_kernel_spmd

dt = mybir.dt
AF = mybir.ActivationFunctionType
ALU = mybir.AluOpType

MAGIC = float(1.5 * 2**23)

D = 1024
F = 4096
L = 4
NCORES = 8
TOK = 1024

TOKT = 256            # tokens per sweep
NSW = TOK // TOKT     # sweeps per layer
NDJ = D // 128        # 8 d-tiles
NFC = F // 128        # 32 f-chunks
NPK = NFC // 4        # 8 f-packs of 4 chunks
NOCT = 4              # wd octets per quad phase
FP8 = dt.float8e4


def _split_excess_waits(nc, max_waits: int = 1) -> int:
    """walrus in this container rejects >1 sync-wait per instruction; split
    extras into standalone event-semaphore waits on the same engine."""
    n = 0
    for func in nc.m.functions:
        for block in func.blocks:
            changed = False
            out = []
            for inst in block.instructions:
                si = getattr(inst, "sync_info", None)
                if si is not None and si.on_wait and len(si.on_wait) > max_waits:
                    waits = list(si.on_wait)
                    for j, w in enumerate(waits[max_waits:]):
                        out.append(
                            mybir.InstEventSemaphore(
                                name=f"{inst.name}-xw{j}",
                                engine=inst.engine,
                                ins=[],
                                outs=[],
                                sync_info=mybir.SyncInfo(on_wait=[w], on_update=[]),
                            )
                        )
                        n += 1
                    inst.sync_info = mybir.SyncInfo(
                        on_wait=waits[:max_waits], on_update=list(si.on_update)
                    )
                    changed = True
                out.append(inst)
            if changed:
                block.instructions = out
    return n


def _b3(ap, n=4):
    """Broadcast a [128, T] AP across a middle dim of n (stride 0)."""
    return bass.AP(ap.tensor, ap.offset, [ap.ap[0], [0, n], ap.ap[1]])


DEBUG = False


def build(n_cores: int = NCORES) -> bass.Bass:
    nc = bass.Bass(num_devices=n_cores)

    x_ext = nc.declare_dram_parameter("x", [D, TOK], dt.float32, isOutput=False)
    rsc_ext = nc.declare_dram_parameter("rsc", [128, L * NDJ], dt.float32, isOutput=False)
    # chunk-grouped layouts: 4 f-chunks (wg/wu) or one quad-octet (wd) per
    # DMA, 4KB contiguous per partition
    wg_ext = nc.declare_dram_parameter("wg", [L, NPK, 128, 4, NDJ, 128], FP8, isOutput=False)
    wu_ext = nc.declare_dram_parameter("wu", [L, NPK, 128, 4, NDJ, 128], FP8, isOutput=False)
    wd_ext = nc.declare_dram_parameter("wd", [L, 2, NOCT, 128, 4, 8, 128], FP8, isOutput=False)
    out_ext = nc.declare_dram_parameter("out", [D, TOK], dt.float32, isOutput=True)
    if DEBUG:
        dbg_ext = nc.declare_dram_parameter("dbg", [16, 128, TOKT], dt.float32, isOutput=True)

    with tile.TileContext(nc) as tc, ExitStack() as ctx:
        P = ctx.enter_context
        const = P(tc.tile_pool(name="const", bufs=1))
        xpool = P(tc.tile_pool(name="X", bufs=1))
        qpool = P(tc.tile_pool(name="q", bufs=5))
        gbpool = P(tc.tile_pool(name="gb", bufs=10))
        gqpool = P(tc.tile_pool(name="gq", bufs=9))
        t4pool = P(tc.tile_pool(name="t4", bufs=4))
        t0pool = P(tc.tile_pool(name="t0", bufs=4))
        trpool = P(tc.tile_pool(name="tr", bufs=4))
        stpool = P(tc.tile_pool(name="st", bufs=1))
        bcpool = P(tc.tile_pool(name="bc", bufs=3))
        smpool = P(tc.tile_pool(name="sm", bufs=2))
        wgupool = P(tc.tile_pool(name="wgu", bufs=2))
        wdpool = P(tc.tile_pool(name="wd", bufs=3))
        psUV = P(tc.tile_pool(name="psUV", bufs=3, space="PSUM"))
        psX = P(tc.tile_pool(name="psX", bufs=1, space="PSUM"))

        # ---------- constants ----------
        nc.gpsimd.load_library(library_config.attn)
        mag = const.tile([128, 1], dt.float32, tag="mag")
        nc.gpsimd.memset(mag[:], MAGIC)
        rscol = const.tile([128, L * NDJ], dt.float32, tag="rscol")
        nc.sync.dma_start(rscol[:], rsc_ext[:, :])

        def sbc_ap(l, dj):
            return rscol[:, l * NDJ + dj:l * NDJ + dj + 1]

        def tap(i, ap):
            if DEBUG:
                nc.sync.dma_start(dbg_ext[i], ap)

        # ---------- persistent X tiles (packed per quad) ----------
        X = {}
        for Q in range(2):
            for s in range(NSW):
                X[(Q, s)] = xpool.tile([128, 4, TOKT], dt.float32,
                                       tag=f"x{Q}_{s}", name=f"x{Q}_{s}")

        # down-proj accumulator: [128, 8, TOKT] fp32 = 4 PSUM banks
        xps = psX.tile([128, NDJ, TOKT], dt.float32, tag="xps", name="xps")

        st = {}  # per (l, s) state

        def S(l, s):
            return st.setdefault((l, s), {})

        # ---------- x load ----------
        def emit_xload(s):
            for dj in range(NDJ):
                nc.sync.dma_start(
                    X[(dj // 4, s)][:, dj % 4, :],
                    x_ext[dj * 128:(dj + 1) * 128, s * TOKT:(s + 1) * TOKT])

        # ---------- stats gather (ssq + max|sbc*x|) from X quad ----------
        def emit_stats_quad(l, s, Q):
            """Accumulate S (sum over d of x^2) and M (max over d of |sbc*x|)
            for layer l's phase-1 stats of sweep s; Q=0 then Q=1."""
            d = S(l, s)
            XQ = X[(Q, s)]
            sq4 = t4pool.tile([128, 4, TOKT], dt.float32, tag="t4", name="sq4")
            nc.vector.tensor_tensor(sq4[:], XQ[:], XQ[:], op=ALU.mult)
            if Q == 0:
                d["S"] = smpool.tile([128, TOKT], dt.float32, tag="S", name="Ssum", bufs=4)
                d["M"] = smpool.tile([128, TOKT], dt.float32, tag="M", name="Mmax", bufs=4)
            for dji in range(4):
                if Q == 0 and dji == 0:
                    nc.vector.tensor_scalar(d["S"][:], sq4[:, 0, :], 0.0, None, op0=ALU.add)
                else:
                    nc.vector.tensor_tensor(d["S"][:], d["S"][:], sq4[:, dji, :], op=ALU.add)
            sxs = []
            for dji in range(4):
                dj = Q * 4 + dji
                sx = trpool.tile([128, TOKT], dt.float32, tag="tr", name="sx")
                nc.scalar.activation(sx[:], XQ[:, dji, :], AF.Abs, scale=sbc_ap(l, dj))
                sxs.append((dj, sx))
            for dj, sx in sxs:
                if dj == 0:
                    nc.vector.tensor_scalar(d["M"][:], sx[:], 0.0, None, op0=ALU.max)
                else:
                    nc.vector.tensor_tensor(d["M"][:], d["M"][:], sx[:], op=ALU.max)

        # ---------- phase 1: rms + act-quant scales ----------
        def emit_phase1(l, s):
            d = S(l, s)
            nc.gpsimd.partition_all_reduce(d["S"][:], d["S"][:], 128, ReduceOp.add)
            nc.gpsimd.partition_all_reduce(d["M"][:], d["M"][:], 128, ReduceOp.max)
            ms = stpool.tile([128, TOKT], dt.float32, tag="st1", name="ms")
            rt = stpool.tile([128, TOKT], dt.float32, tag="st2", name="rt")
            rstd = stpool.tile([128, TOKT], dt.float32, tag="st3", name="rstd")
            nwt = stpool.tile([128, TOKT], dt.float32, tag="st4", name="nwt")
            nc.vector.tensor_scalar(ms[:], d["S"][:], 1.0 / D, 1e-6, op0=ALU.mult, op1=ALU.add)
            nc.scalar.activation(rt[:], ms[:], AF.Sqrt)
            nc.vector.reciprocal(rstd[:], rt[:])
            # Newton step refines the Sqrt-LUT rsqrt (v8-proven)
            nc.vector.tensor_tensor(nwt[:], rstd[:], rstd[:], op=ALU.mult)
            nc.vector.tensor_tensor(nwt[:], nwt[:], ms[:], op=ALU.mult)
            nc.vector.tensor_scalar(nwt[:], nwt[:], -0.5, 1.5, op0=ALU.mult, op1=ALU.add)
            nc.vector.tensor_tensor(rstd[:], rstd[:], nwt[:], op=ALU.mult)
            maxh = stpool.tile([128, TOKT], dt.float32, tag="st5", name="maxh")
            sr = stpool.tile([128, TOKT], dt.float32, tag="st6", name="sr")
            nc.vector.tensor_tensor(maxh[:], d["M"][:], rstd[:], op=ALU.mult)
            nc.vector.tensor_scalar(maxh[:], maxh[:], 1e-5, None, op0=ALU.add)
            nc.vector.reciprocal(sr[:], maxh[:])
            nc.vector.tensor_scalar(sr[:], sr[:], 127.0, 1e3, op0=ALU.mult, op1=ALU.min)
            nc.vector.tensor_scalar(sr[:], sr[:], 1e-3, None, op0=ALU.max)
            c1 = bcpool.tile([128, TOKT], dt.float32, tag="c1", name="c1bc")
            rs = bcpool.tile([128, TOKT], dt.float32, tag="rs", name="rsbc")
            nc.vector.tensor_tensor(c1[:], sr[:], rstd[:], op=ALU.mult)
            # rs = 1/s recip-free: clip(maxh/127, 1e-3, 1e3) (maxh incl +1e-5)
            nc.vector.tensor_scalar(rs[:], maxh[:], 1.0 / 127.0, 1e-3, op0=ALU.mult, op1=ALU.max)
            nc.vector.tensor_scalar(rs[:], rs[:], 1e3, None, op0=ALU.min)
            d["c1"], d["rs"] = c1, rs
            if (l, s) == (0, 0):
                tap(0, d["S"][:]); tap(1, d["M"][:]); tap(2, c1[:]); tap(3, rs[:])

        # ---------- q pass (packed per quad; A = mult+magic1, B = round) ----------
        def emit_qpass_a(l, s):
            d = S(l, s)
            t04s = []
            for Q in range(2):
                t04 = t4pool.tile([128, 4, TOKT], dt.float32, tag="qt4", name="t04", bufs=4)
                nc.vector.tensor_tensor(t04[:], X[(Q, s)][:], _b3(d["c1"][:]), op=ALU.mult)
                for dji in range(4):
                    nc.scalar.activation(t04[:, dji, :], t04[:, dji, :], AF.Identity,
                                         scale=sbc_ap(l, Q * 4 + dji), bias=mag[:])
                t04s.append(t04)
            d["t04"] = t04s

        def emit_qpass_b(l, s):
            d = S(l, s)
            qp = []
            for Q in range(2):
                q4 = qpool.tile([128, 4, TOKT], dt.bfloat16, tag="q", name="q4")
                nc.vector.tensor_scalar(q4[:], d["t04"][Q][:], -MAGIC, None, op0=ALU.add)
                qp.append(q4)
            del d["t04"]
            d["qp"] = qp

        def emit_qpass(l, s):
            emit_qpass_a(l, s)
            emit_qpass_b(l, s)

        def q_ap(d, dk):
            return d["qp"][dk // 4][:, dk % 4, :]

        # ---------- up-proj + GLU, one f-chunk (emission-pipelined) ----------
        def flush_gb(d, l, s, keep=0):
            """Emit deferred V drains (gb = silu * V); lag keeps DVE head from
            waiting on the scalar silu round-trip."""
            pend = d.setdefault("pend", [])
            while len(pend) > keep:
                c, uv, t0 = pend.pop(0)
                cg, ci = divmod(c, 4)
                gbp = d["gbp"][cg]
                nc.vector.tensor_tensor(gbp[:, ci, :], t0[:], uv[:, 1, :], op=ALU.mult)
                if ci == 3:
                    ab4 = t4pool.tile([128, 4, TOKT], dt.float32, tag="t4", name="ab4")
                    nc.scalar.activation(ab4[:], gbp[:], AF.Abs)
                    d.setdefault("pendab", []).append((cg, ab4))

        def flush_mx(d, keep=0):
            pendab = d.setdefault("pendab", [])
            while len(pendab) > keep:
                cg, ab4 = pendab.pop(0)
                if cg == 0:
                    d["mx4"] = smpool.tile([128, 4, TOKT], dt.float32, tag="mx4", name="mx4")
                    nc.vector.tensor_scalar(d["mx4"][:], ab4[:], 0.0, None, op0=ALU.max)
                else:
                    nc.vector.tensor_tensor(d["mx4"][:], d["mx4"][:], ab4[:], op=ALU.max)

        def emit_uv_chunk(l, s, c):
            d = S(l, s)
            cg, ci = divmod(c, 4)
            wgu = d.setdefault("wgu", {})
            if cg not in wgu:
                wgt4 = wgupool.tile([128, 4, NDJ, 128], FP8, tag="wg", name="wgt")
                wut4 = wgupool.tile([128, 4, NDJ, 128], FP8, tag="wu", name="wut")
                nc.sync.dma_start(wgt4[:], wg_ext[l, cg])
                nc.sync.dma_start(wut4[:], wu_ext[l, cg])
                wgu[cg] = (wgt4, wut4)
            wgt4, wut4 = wgu[cg]
            wgt = wgt4[:, ci]
            wut = wut4[:, ci]
            uv = psUV.tile([128, 2, TOKT], dt.float32, tag="uv", name="uvps")
            # U and V share one 2KB PSUM bank; start=True zeroes the WHOLE
            # bank, so only U's first matmul sets it.
            for dk in range(NDJ):
                nc.tensor.matmul(uv[:, 0, :], wgt[:, dk, :], q_ap(d, dk),
                                 start=(dk == 0), stop=(dk == NDJ - 1))
                nc.tensor.matmul(uv[:, 1, :], wut[:, dk, :], q_ap(d, dk),
                                 start=False, stop=(dk == NDJ - 1))
            if c % 4 == 0:
                d.setdefault("gbp", []).append(
                    gbpool.tile([128, 4, TOKT], dt.float32, tag="gb", name="gbp"))
            t0 = t0pool.tile([128, TOKT], dt.float32, tag="t0", name="silut")
            nc.vector.tensor_tensor(t0[:], uv[:, 0, :], d["rs"][:], op=ALU.mult)
            nc.scalar.activation(t0[:], t0[:], AF.Silu)
            d.setdefault("pend", []).append((c, uv, t0))
            flush_gb(d, l, s, keep=2)
            flush_mx(d, keep=1)

        # ---------- phase 2: g quant scales ----------
        def emit_phase2(l, s):
            d = S(l, s)
            flush_gb(d, l, s, keep=0)
            flush_mx(d, keep=0)
            mx = stpool.tile([128, TOKT], dt.float32, tag="st1", name="mxf")
            nc.vector.tensor_tensor(mx[:], d["mx4"][:, 0, :], d["mx4"][:, 1, :], op=ALU.max)
            nc.vector.tensor_tensor(mx[:], mx[:], d["mx4"][:, 2, :], op=ALU.max)
            nc.vector.tensor_tensor(mx[:], mx[:], d["mx4"][:, 3, :], op=ALU.max)
            nc.gpsimd.partition_all_reduce(mx[:], mx[:], 128, ReduceOp.max)
            g1 = stpool.tile([128, TOKT], dt.float32, tag="g1", name="g1")
            g2 = stpool.tile([128, TOKT], dt.float32, tag="g2", name="g2")
            nc.vector.tensor_tensor(g1[:], mx[:], d["rs"][:], op=ALU.mult)
            nc.vector.tensor_scalar(g1[:], g1[:], 1e-5, None, op0=ALU.add)
            # rs2p = 1/s2 computed recip-free: clip((g1+1e-5)/127, 1e-3, 1e3)
            rs2p = bcpool.tile([128, TOKT], dt.float32, tag="rs2", name="rs2p")
            nc.vector.tensor_scalar(rs2p[:], g1[:], 1.0 / 127.0, 1e-3, op0=ALU.mult, op1=ALU.max)
            nc.vector.tensor_scalar(rs2p[:], rs2p[:], 1e3, None, op0=ALU.min)
            nc.vector.reciprocal(g2[:], g1[:])
            nc.vector.tensor_scalar(g2[:], g2[:], 127.0, 1e3, op0=ALU.mult, op1=ALU.min)
            nc.vector.tensor_scalar(g2[:], g2[:], 1e-3, None, op0=ALU.max)  # = s2
            c2p = bcpool.tile([128, TOKT], dt.float32, tag="c2", name="c2p")
            nc.vector.tensor_tensor(c2p[:], g2[:], d["rs"][:], op=ALU.mult)
            d["c2p"], d["rs2p"] = c2p, rs2p
            if (l, s) == (0, 0):
                tap(7, mx[:]); tap(8, c2p[:]); tap(9, rs2p[:])

        # ---------- gq rounding (packed, fused DVE magic) ----------
        def emit_gq(l, s):
            d = S(l, s)
            gqp = []
            for pk in range(NPK):
                gbp = d["gbp"][pk]
                nc.vector.tensor_tensor(gbp[:], gbp[:], _b3(d["c2p"][:]), op=ALU.mult)
                g4 = gqpool.tile([128, 4, TOKT], dt.bfloat16, tag="gq", name="gq4")
                nc.vector.tensor_scalar(g4[:], gbp[:], MAGIC, -MAGIC, op0=ALU.add, op1=ALU.add)
                gqp.append(g4)
            d["gqp"] = gqp

        def gq_ap(d, ft):
            return d["gqp"][ft // 4][:, ft % 4, :]

        # ---------- down-proj + residual + next-layer stats ----------
        def emit_wd_oct(l, Q, oct):
            t = wdpool.tile([128, 4, 8, 128], FP8, tag="wd", name="wdt")
            nc.sync.dma_start(t[:], wd_ext[l, Q, oct])
            return t

        def emit_down(l, s, pre_oct0):
            d = S(l, s)
            for Q in range(2):
                for oct in range(NOCT):
                    wdt = pre_oct0 if (Q == 0 and oct == 0) else emit_wd_oct(l, Q, oct)
                    for k in range(8):
                        ft = oct * 8 + k
                        for dji in range(4):
                            dj = Q * 4 + dji
                            # adjacent dj pairs share a PSUM bank: even dj's
                            # first matmul zeroes it, odd dj rides the zero
                            nc.tensor.matmul(
                                xps[:, dj, :], wdt[:, dji, k, :], gq_ap(d, ft),
                                start=(ft == 0 and dj % 2 == 0), stop=(ft == NFC - 1))
                # drain quad: X += xps * rs2p  (batched, bcast AP)
                upd4 = t4pool.tile([128, 4, TOKT], dt.float32, tag="t4", name="upd4")
                nc.vector.tensor_tensor(upd4[:], xps[:, Q * 4:Q * 4 + 4, :],
                                        _b3(d["rs2p"][:]), op=ALU.mult)
                if (s, Q) == (0, 0) and l == 0:
                    tap(10, upd4[:, 0, :])
                nc.vector.tensor_tensor(X[(Q, s)][:], X[(Q, s)][:], upd4[:], op=ALU.add)
                if (s, Q) == (0, 0) and l < 3:
                    tap(11 + l, X[(0, s)][:, 0, :])
                if l + 1 < L:
                    emit_stats_quad(l + 1, s, Q)
                else:
                    for dji in range(4):
                        dj = Q * 4 + dji
                        nc.sync.dma_start(
                            out_ext[dj * 128:(dj + 1) * 128, s * TOKT:(s + 1) * TOKT],
                            X[(Q, s)][:, dji, :])

        # ---------- main ----------
        for s in range(NSW):
            emit_xload(s)
        emit_stats_quad(0, 0, 0)
        emit_stats_quad(0, 0, 1)
        emit_phase1(0, 0)
        emit_qpass(0, 0)
        for s in range(1, NSW):
            emit_stats_quad(0, s, 0)
            emit_stats_quad(0, s, 1)

        for l in range(L):
            for s in range(NSW):
                first = (l == 0 and s == 0)
                nl, ns = (l, s + 1) if s + 1 < NSW else (l + 1, 0)
                for c in range(0 if first else 8, NFC):
                    emit_uv_chunk(l, s, c)
                    if c == 10 and nl < L:
                        emit_phase1(nl, ns)
                    if c == 16 and nl < L:
                        emit_qpass_a(nl, ns)
                    if c == 24 and nl < L:
                        emit_qpass_b(nl, ns)
                emit_phase2(l, s)
                pre_oct0 = emit_wd_oct(l, 0, 0)
                emit_gq(l, s)
                # hide phase-2 + gq latency under pre-issued chunks of next sweep
                if nl < L:
                    for cpre in range(8):
                        emit_uv_chunk(nl, ns, cpre)
                emit_down(l, s, pre_oct0)
                st.pop((l, s), None)

    library_overlay.lower_extended_insts(nc)
    _split_excess_waits(nc)
    return nc


_nc_cache = {}


def _get_nc(key=(NCORES,)):
    if key not in _nc_cache:
        _nc_cache[key] = build(*key)
    return _nc_cache[key]


def _ternarize(w):
    """Exact host-side ternarize: sign(round(w * 127/(max|w|+1e-5))) in
    fp32 RNE, matching ternarize(weight_quant(w)) in the reference."""
    w = np.ascontiguousarray(w, dtype=np.float32)
    m = np.float32(np.abs(w).max())
    s = np.float32(127.0) / (m + np.float32(1e-5))
    t = np.round(w * s)
    return np.clip(t, np.float32(-1.0), np.float32(1.0))


def _pack_weights(wg, wu, wd):
    f8 = ml_dtypes.float8_e4m3
    wg_p = np.empty((L, NPK, 128, 4, NDJ, 128), dtype=f8)
    wu_p = np.empty((L, NPK, 128, 4, NDJ, 128), dtype=f8)
    wd_p = np.empty((L, 2, NOCT, 128, 4, 8, 128), dtype=f8)
    for l in range(L):
        tg = _ternarize(wg[l])   # [D, F]
        tu = _ternarize(wu[l])
        td = _ternarize(wd[l])   # [F, D]
        # [dk, p, cg, ci, m] -> [cg, p, ci, dk, m]
        wg_p[l] = tg.reshape(NDJ, 128, NPK, 4, 128).transpose(2, 1, 3, 0, 4).astype(f8)
        wu_p[l] = tu.reshape(NDJ, 128, NPK, 4, 128).transpose(2, 1, 3, 0, 4).astype(f8)
        # [oct, k, p, Q, dji, m] -> [Q, oct, p, dji, k, m]
        wd_p[l] = td.reshape(NOCT, 8, 128, 2, 4, 128).transpose(3, 0, 2, 4, 1, 5).astype(f8)
    return np.ascontiguousarray(wg_p), np.ascontiguousarray(wu_p), np.ascontiguousarray(wd_p)


def _make_in_maps(x, rs, wg, wu, wd, n_cores=NCORES):
    wg_p, wu_p, wd_p = _pack_weights(wg, wu, wd)
    # rscol[p, l*8+dk] = rms_scale[l, dk*128+p]
    rsc = np.ascontiguousarray(
        rs.reshape(L, NDJ, 128).transpose(2, 0, 1).reshape(128, L * NDJ),
        dtype=np.float32)
    in_maps = []
    for c in range(n_cores):
        in_maps.append({
            "x": np.ascontiguousarray(x[c].T),   # [D, TOK]
            "rsc": rsc,
            "wg": wg_p,
            "wu": wu_p,
            "wd": wd_p,
        })
    return in_maps


def kernel(x, rms_scale, W_g, W_u, W_d):
    """Full-input entry point: shard over batch, run 8-core SPMD, gather."""
    x = np.ascontiguousarray(np.asarray(x, dtype=np.float32))
    rs = np.ascontiguousarray(np.asarray(rms_scale, dtype=np.float32))
    wg = np.ascontiguousarray(np.asarray(W_g, dtype=np.float32))
    wu = np.ascontiguousarray(np.asarray(W_u, dtype=np.float32))
    wd = np.ascontiguousarray(np.asarray(W_d, dtype=np.float32))
    B, Sx, Dx = x.shape
    assert (B, Sx, Dx) == (NCORES, TOK, D), (B, Sx, Dx)
    nc = _get_nc()
    in_maps = _make_in_maps(x, rs, wg, wu, wd)
    res = run_bass_kernel_spmd(nc, in_maps, list(range(NCORES)))
    return np.stack([np.ascontiguousarray(res.results[c]["out"].T)
                     for c in range(NCORES)], axis=0)
